# revision 43
# baseline (speedup 1.0000x reference)
"""Trainium2 Bass kernel for 16-head causal MHA + output projection.

Sharding: tensor-parallel over the head axis. 16 heads / 8 cores = 2 heads
per core. Each core runs causal flash-style attention for its 2 heads fully
on-chip, then multiplies its 128-wide slice of the concatenated context with
the matching 128 columns of Wo (row-parallel linear). The host gather sums
the 8 partial projections (the TP all-reduce realized as the unshard step)
and adds the bias.

Device algorithm per (head, batch) pair, per 512-wide query block j:
  S^T[s, t]   = sum_d K[s,d] Q[t,d]                  (PE, K=64 contraction)
  E           = exp(S^T * 0.125)                      (ACT, direct from PSUM;
                 no max subtraction -- scores are ~N(0,1), max ~6)
  causal mask: E[s,t] = 0 where s > t                 (0/1 triangular multiply
                 on boundary 128-tiles; fully-masked regions are simply never
                 computed/consumed)
  ctx^T[d, t] = sum_s V'[s,d] E[s,t]                  (PE, K=128; V' has a
                 ones column so row 64 of the output is the softmax denom)
  ctx^T[0:64] /= denom                                (DVE reciprocal + mul)
Then out_partial[t, :] = ctx^T[:, t].T @ WoT          (PE, K=128)

Q^T, K^T, V' and Wo^T slices are prepared on the host so the device never
transposes anything.
"""

import os
import sys

import numpy as np

try:
    import concourse.bass as bass  # noqa: F401
except ImportError:
    sys.path.insert(0, "/opt/trn_rl_repo")

import concourse.bass as bass
import concourse.tile as tile
from concourse import bacc, bass_utils, mybir

# Problem constants (hardcoded per contract)
H, B, T, DH = 16, 2, 2048, 64
MODEL_DIM = 1024
N_CORES = 8
HPC = H // N_CORES  # heads per core = 2
NPAIRS = HPC * B  # (head, batch) pairs per core = 4
TBLK = 512  # query-block width
STILE = 128  # key-block (s) tile height
CHUNK = 2  # s-tiles per PSUM chunk / exp instruction

F32 = mybir.dt.float32
F32R = mybir.dt.float32r
BF16 = mybir.dt.bfloat16

# "f32"  : everything fp32 (PE runs fp32 matmul at 1/4 rate — slow, exact)
# "f32r" : fp32 storage, matmuls issued as float32r (full rate, reduced mul precision)
# "bf16" : q/k/v/expS in bf16 (full rate, halved DMA), Wo matmul as f32r
MODE = os.environ.get("MHA_MODE", "f32r")


def _build_program(mode=MODE):
    nc = bacc.Bacc(
        "TRN2",
        debug=False,
        num_devices=N_CORES,
        name="mha_tp",
    )

    if mode == "bf16":
        qkv_dt = ex_dt = ctx_dt = BF16
    elif mode == "f32r":
        qkv_dt = ex_dt = ctx_dt = F32R
    else:
        qkv_dt = ex_dt = ctx_dt = F32

    # pair-packed layouts: group g = batch b holds head0 in partitions 0-63,
    # head1 in partitions 64-127 (full-partition DMAs run 2x faster)
    qT_d = nc.dram_tensor("qT", [B, HPC * DH, T], qkv_dt, kind="ExternalInput").ap()
    kT_d = nc.dram_tensor("kT", [B, HPC * DH, T], qkv_dt, kind="ExternalInput").ap()
    vp_d = nc.dram_tensor(
        "vp", [B, 128, HPC * (T // 128), DH + 1], qkv_dt, kind="ExternalInput"
    ).ap()
    woT_d = nc.dram_tensor(
        "woT", [HPC * DH, MODEL_DIM], ctx_dt, kind="ExternalInput"
    ).ap()
    tri_d = nc.dram_tensor("tri", [STILE, STILE], ex_dt, kind="ExternalInput").ap()
    out_d = nc.dram_tensor("out", [B * T, MODEL_DIM], F32, kind="ExternalOutput").ap()

    n_tblk = T // TBLK  # 4

    with tile.TileContext(nc) as tc:
        with (
            tc.tile_pool(name="const", bufs=1) as const,
            tc.tile_pool(name="io", bufs=2) as io,
            tc.tile_pool(name="exp", bufs=6) as expool,
            tc.tile_pool(name="ctx", bufs=1) as ctxpool,
            tc.tile_pool(name="norm", bufs=4) as normpool,
            tc.tile_pool(name="outp", bufs=6) as outpool,
            tc.tile_pool(name="sps", bufs=2, space="PSUM") as spspool,
            tc.tile_pool(name="pvp", bufs=2, space="PSUM") as pvpool,
            tc.tile_pool(name="wop", bufs=2, space="PSUM") as wopool,
        ):
            woT_sb = const.tile([HPC * DH, MODEL_DIM], ctx_dt)
            tri_sb = const.tile([STILE, STILE], ex_dt)
            zbias = const.tile([128, 1], F32)
            nc.vector.memset(zbias, 0.0)

            ctx_sb = ctxpool.tile([HPC * DH, B * T], ctx_dt)
            pending_wo = []

            def _wo_block(j, b):
                for r in range(j * 4, (j + 1) * 4):
                    colr = b * T + r * 128
                    for ob in range(MODEL_DIM // 512):
                        wop = wopool.tile([128, 512], F32, tag="wop")
                        nc.tensor.matmul(
                            wop,
                            lhsT=ctx_sb[:, colr : colr + 128],
                            rhs=woT_sb[:, ob * 512 : (ob + 1) * 512],
                            start=True,
                            stop=True,
                        )
                        ot = outpool.tile([128, 512], F32, tag="ot")
                        nc.vector.tensor_copy(ot, wop)
                        nc.sync.dma_start(
                            out=out_d[
                                b * T + r * 128 : b * T + (r + 1) * 128,
                                ob * 512 : (ob + 1) * 512,
                            ],
                            in_=ot,
                        )

            def _load_batch(b):
                # first slices are small so the first S-matmul starts early
                qT_sb = io.tile([HPC * DH, T], qkv_dt, tag="qT", name=f"qT{b}")
                kT_sb = io.tile([HPC * DH, T], qkv_dt, tag="kT", name=f"kT{b}")
                splits = [0, T // 4, T // 2, T] if b == 0 else [0, T]
                for lo2, hi2 in zip(splits[:-1], splits[1:]):
                    cols = slice(lo2, hi2)
                    nc.sync.dma_start(out=kT_sb[:, cols], in_=kT_d[b][:, cols])
                    nc.sync.dma_start(out=qT_sb[:, cols], in_=qT_d[b][:, cols])
                vp_sb = io.tile(
                    [128, HPC * (T // 128), DH + 1], qkv_dt, tag="vp", name=f"vp{b}"
                )
                if b == 0:
                    # head0's first s-tiles land first so PV(j=0) starts early
                    nc.sync.dma_start(out=vp_sb[:, 0:4, :], in_=vp_d[b][:, 0:4, :])
                    nc.sync.dma_start(out=tri_sb, in_=tri_d)
                    nc.sync.dma_start(out=vp_sb[:, 4:, :], in_=vp_d[b][:, 4:, :])
                    nc.sync.dma_start(out=woT_sb, in_=woT_d)
                else:
                    nc.sync.dma_start(out=vp_sb, in_=vp_d[b])
                return qT_sb, kT_sb, vp_sb

            loaded = {0: _load_batch(0)}
            for b in range(B):
                qT_sb, kT_sb, vp_sb = loaded.pop(b)

                for hl in range(HPC):
                    bp = hl * DH  # partition base of this head's qT/kT rows
                    for j in range(n_tblk):
                        final_j = hl == HPC - 1 and b == B - 1 and j == n_tblk - 1
                        pv = pvpool.tile([DH + 1, TBLK], F32, tag="pv")
                        n_stiles = (j + 1) * (TBLK // STILE)
                        for c in range(0, n_stiles, CHUNK):
                            if final_j and c == CHUNK:
                                # flush deferred projections behind the final
                                # block's first S-chunk so ACT isn't starved
                                while pending_wo:
                                    pending_wo.pop(0)()
                            diag0 = j * (TBLK // STILE)  # first diagonal s-tile
                            # deep-diagonal chunk (tiles >= half-masked): narrow
                            # the S matmul + exp to the causally needed columns
                            deep = c >= diag0 + 2
                            sps = spspool.tile([128, CHUNK * TBLK], F32, tag="sps")
                            for m in range(CHUNK):
                                k = c + m
                                lo = (k - diag0) * STILE if deep else 0
                                nc.tensor.matmul(
                                    sps[:, m * TBLK + lo : (m + 1) * TBLK],
                                    lhsT=kT_sb[
                                        bp : bp + DH, k * STILE : (k + 1) * STILE
                                    ],
                                    rhs=qT_sb[
                                        bp : bp + DH, j * TBLK + lo : (j + 1) * TBLK
                                    ],
                                    start=True,
                                    stop=True,
                                    tile_position=(bp, 0),
                                )
                            ex = expool.tile([128, CHUNK * TBLK], ex_dt, tag="ex")
                            if deep:
                                for m in range(CHUNK):
                                    k = c + m
                                    lo = (k - diag0) * STILE
                                    nc.scalar.activation(
                                        ex[:, m * TBLK + lo : (m + 1) * TBLK],
                                        sps[:, m * TBLK + lo : (m + 1) * TBLK],
                                        func=mybir.ActivationFunctionType.Exp,
                                        bias=zbias,
                                        scale=0.125,
                                    )
                            else:
                                nc.scalar.activation(
                                    ex,
                                    sps,
                                    func=mybir.ActivationFunctionType.Exp,
                                    bias=zbias,
                                    scale=0.125,
                                )
                            for m in range(CHUNK):
                                k = c + m
                                if k >= diag0:
                                    # boundary 128-col gets the triangular mask;
                                    # fully-masked cols to its left are never
                                    # consumed (PV below starts at the boundary)
                                    mm = k - diag0
                                    exs = ex[:, m * TBLK : (m + 1) * TBLK]
                                    nc.gpsimd.tensor_mul(
                                        exs[:, mm * STILE : (mm + 1) * STILE],
                                        exs[:, mm * STILE : (mm + 1) * STILE],
                                        tri_sb,
                                    )
                            for m in range(CHUNK):
                                k = c + m
                                lo = (k - diag0) * STILE if k > diag0 else 0
                                nc.tensor.matmul(
                                    pv[:, lo:],
                                    lhsT=vp_sb[:, hl * (T // 128) + k, :],
                                    rhs=ex[:, m * TBLK + lo : (m + 1) * TBLK],
                                    start=(k == 0),
                                    stop=(k == n_stiles - 1),
                                    skip_group_check=True,
                                )

                        col = b * T + j * TBLK
                        last_block = (
                            hl == HPC - 1 and b == B - 1 and j == n_tblk - 1
                        )
                        if not last_block:
                            # normalize: ctx^T[d, t] = pv[d, t] / pv[64, t]
                            rc = normpool.tile([1, TBLK], F32, tag="rc")
                            nc.vector.reciprocal(rc, pv[DH : DH + 1, :])
                            rcb = normpool.tile([DH, TBLK], F32, tag="rcb")
                            nc.gpsimd.partition_broadcast(out_ap=rcb, in_ap=rc)
                            nc.vector.tensor_mul(
                                ctx_sb[hl * DH : (hl + 1) * DH, col : col + TBLK],
                                pv[0:DH, :],
                                rcb,
                            )
                        if hl == 0 and j == n_tblk - 1 and b + 1 < B:
                            # prefetch next batch's inputs now: their SP triggers
                            # queue ahead of this batch's output-DMA sem-waits
                            loaded[b + 1] = _load_batch(b + 1)
                        if hl == HPC - 1 and not last_block:
                            # emit the projection AFTER the next j's attention so
                            # the PE prioritizes S-matmuls (ACT's dependency)
                            def _emit_wo(j=j, b=b):
                                _wo_block(j, b)

                            pending_wo.append(_emit_wo)
                            if len(pending_wo) > 1:
                                pending_wo.pop(0)()
                        if last_block:
                            # drain deferred projections, then finish the final
                            # block at 128-col granularity so the tail pipeline
                            # (normalize -> Wo -> copy -> DMA) is 4x finer
                            while pending_wo:
                                pending_wo.pop(0)()
                            for pc in range(TBLK // STILE):
                                psl = slice(pc * STILE, (pc + 1) * STILE)
                                cp = col + pc * STILE
                                rc = normpool.tile([1, STILE], F32, tag="rcp")
                                nc.vector.reciprocal(rc, pv[DH : DH + 1, psl])
                                rcb = normpool.tile([DH, STILE], F32, tag="rcbp")
                                nc.gpsimd.partition_broadcast(out_ap=rcb, in_ap=rc)
                                nc.vector.tensor_mul(
                                    ctx_sb[hl * DH : (hl + 1) * DH, cp : cp + STILE],
                                    pv[0:DH, psl],
                                    rcb,
                                )
                                r = j * 4 + pc
                                for ob in range(MODEL_DIM // 512):
                                    wop = wopool.tile([128, 512], F32, tag="wop")
                                    nc.tensor.matmul(
                                        wop,
                                        lhsT=ctx_sb[:, cp : cp + STILE],
                                        rhs=woT_sb[:, ob * 512 : (ob + 1) * 512],
                                        start=True,
                                        stop=True,
                                    )
                                    ot = outpool.tile([128, 512], F32, tag="ot")
                                    if ob % 2 == 1:
                                        nc.scalar.copy(ot, wop)
                                    else:
                                        nc.vector.tensor_copy(ot, wop)
                                    nc.sync.dma_start(
                                        out=out_d[
                                            b * T + r * 128 : b * T + (r + 1) * 128,
                                            ob * 512 : (ob + 1) * 512,
                                        ],
                                        in_=ot,
                                    )

            for f in pending_wo:
                f()

    nc.compile()
    return nc


def make_core_inputs(q, k, v, Wo_w, mode=MODE):
    """Host-side sharding: per-core input dict list."""
    if mode == "bf16":
        import ml_dtypes

        io_dt = ml_dtypes.bfloat16
    else:
        io_dt = np.float32
    q = np.asarray(q, dtype=np.float32)
    k = np.asarray(k, dtype=np.float32)
    v = np.asarray(v, dtype=np.float32)
    Wo_w = np.asarray(Wo_w, dtype=np.float32)
    # multiplicative causal mask for a boundary tile: allow[s, t] = (s <= t)
    tri = np.triu(np.ones((STILE, STILE), dtype=io_dt), 0)
    in_maps = []
    for c in range(N_CORES):
        qT = np.empty((B, HPC * DH, T), dtype=io_dt)
        kT = np.empty((B, HPC * DH, T), dtype=io_dt)
        vp = np.empty((B, 128, HPC * (T // 128), DH + 1), dtype=io_dt)
        for b in range(B):
            for hl in range(HPC):
                h = HPC * c + hl
                qT[b, hl * DH : (hl + 1) * DH] = q[h, b].T
                kT[b, hl * DH : (hl + 1) * DH] = k[h, b].T
                vr = v[h, b].reshape(T // 128, 128, DH).transpose(1, 0, 2)
                sl = slice(hl * (T // 128), (hl + 1) * (T // 128))
                vp[b, :, sl, :DH] = vr
                vp[b, :, sl, DH] = 1.0
        woT = np.ascontiguousarray(Wo_w[:, 128 * c : 128 * (c + 1)].T)
        in_maps.append(
            {"qT": qT, "kT": kT, "vp": vp, "woT": woT, "tri": tri.copy()}
        )
    return in_maps


_NC_CACHE = {}


def get_program(mode=MODE):
    if mode not in _NC_CACHE:
        _NC_CACHE[mode] = _build_program(mode)
    return _NC_CACHE[mode]


def kernel(q, k, v, Wo_w, Wo_b, _trace=False, _trace_kwargs=None, mode=MODE):
    in_maps = make_core_inputs(q, k, v, Wo_w, mode)
    nc = get_program(mode)
    res = bass_utils.run_bass_kernel_spmd(
        nc,
        in_maps,
        core_ids=list(range(N_CORES)),
        trace=_trace,
        **(_trace_kwargs or {}),
    )
    total = np.zeros((B * T, MODEL_DIM), dtype=np.float32)
    for r in res.results:
        total += r["out"]
    total += np.asarray(Wo_b, dtype=np.float32)[None, :]
    out = total.reshape(B, T, MODEL_DIM)
    if _trace:
        return out, res
    return out


# revision 44
# speedup vs baseline: 1.0333x; 1.0333x over previous
"""Trainium2 Bass kernel for 16-head causal MHA + output projection.

Sharding: tensor-parallel over the head axis. 16 heads / 8 cores = 2 heads
per core. Each core runs causal flash-style attention for its 2 heads fully
on-chip, then multiplies its 128-wide slice of the concatenated context with
the matching 128 columns of Wo (row-parallel linear). The host gather sums
the 8 partial projections (the TP all-reduce realized as the unshard step)
and adds the bias.

Device algorithm per (head, batch) pair, per 512-wide query block j:
  S^T[s, t]   = sum_d K[s,d] Q[t,d]                  (PE, K=64 contraction)
  E           = exp(S^T * 0.125)                      (ACT, direct from PSUM;
                 no max subtraction -- scores are ~N(0,1), max ~6)
  causal mask: E[s,t] = 0 where s > t                 (0/1 triangular multiply
                 on boundary 128-tiles; fully-masked regions are simply never
                 computed/consumed)
  ctx^T[d, t] = sum_s V'[s,d] E[s,t]                  (PE, K=128; V' has a
                 ones column so row 64 of the output is the softmax denom)
  ctx^T[0:64] /= denom                                (DVE reciprocal + mul)
Then out_partial[t, :] = ctx^T[:, t].T @ WoT          (PE, K=128)

Q^T, K^T, V' and Wo^T slices are prepared on the host so the device never
transposes anything.
"""

import os
import sys

import numpy as np

try:
    import concourse.bass as bass  # noqa: F401
except ImportError:
    sys.path.insert(0, "/opt/trn_rl_repo")

import concourse.bass as bass
import concourse.tile as tile
from concourse import bacc, bass_utils, mybir

# Problem constants (hardcoded per contract)
H, B, T, DH = 16, 2, 2048, 64
MODEL_DIM = 1024
N_CORES = 8
HPC = H // N_CORES  # heads per core = 2
NPAIRS = HPC * B  # (head, batch) pairs per core = 4
TBLK = 512  # query-block width
STILE = 128  # key-block (s) tile height
CHUNK = 2  # s-tiles per PSUM chunk / exp instruction

F32 = mybir.dt.float32
F32R = mybir.dt.float32r
BF16 = mybir.dt.bfloat16

# "f32"  : everything fp32 (PE runs fp32 matmul at 1/4 rate — slow, exact)
# "f32r" : fp32 storage, matmuls issued as float32r (full rate, reduced mul precision)
# "bf16" : q/k/v/expS in bf16 (full rate, halved DMA), Wo matmul as f32r
MODE = os.environ.get("MHA_MODE", "f32r")


def _build_program(mode=MODE):
    nc = bacc.Bacc(
        "TRN2",
        debug=False,
        num_devices=N_CORES,
        name="mha_tp",
    )

    if mode == "bf16":
        qkv_dt = ex_dt = ctx_dt = BF16
    elif mode == "f32r":
        qkv_dt = ex_dt = ctx_dt = F32R
    else:
        qkv_dt = ex_dt = ctx_dt = F32

    # pair-packed layouts: group g = batch b holds head0 in partitions 0-63,
    # head1 in partitions 64-127 (full-partition DMAs run 2x faster)
    qT_d = nc.dram_tensor("qT", [B, HPC * DH, T], qkv_dt, kind="ExternalInput").ap()
    kT_d = nc.dram_tensor("kT", [B, HPC * DH, T], qkv_dt, kind="ExternalInput").ap()
    vp_d = nc.dram_tensor(
        "vp", [B, 128, HPC * (T // 128), DH + 1], qkv_dt, kind="ExternalInput"
    ).ap()
    woT_d = nc.dram_tensor(
        "woT", [HPC * DH, MODEL_DIM], ctx_dt, kind="ExternalInput"
    ).ap()
    tri_d = nc.dram_tensor("tri", [STILE, STILE], ex_dt, kind="ExternalInput").ap()
    out_d = nc.dram_tensor("out", [B * T, MODEL_DIM], F32, kind="ExternalOutput").ap()

    n_tblk = T // TBLK  # 4

    with tile.TileContext(nc) as tc:
        with (
            tc.tile_pool(name="const", bufs=1) as const,
            tc.tile_pool(name="io", bufs=2) as io,
            tc.tile_pool(name="exp", bufs=6) as expool,
            tc.tile_pool(name="ctx", bufs=1) as ctxpool,
            tc.tile_pool(name="norm", bufs=4) as normpool,
            tc.tile_pool(name="outp", bufs=6) as outpool,
            tc.tile_pool(name="sps", bufs=2, space="PSUM") as spspool,
            tc.tile_pool(name="pvp", bufs=2, space="PSUM") as pvpool,
            tc.tile_pool(name="wop", bufs=2, space="PSUM") as wopool,
        ):
            woT_sb = const.tile([HPC * DH, MODEL_DIM], ctx_dt)
            tri_sb = const.tile([STILE, STILE], ex_dt)
            zbias = const.tile([128, 1], F32)
            nc.vector.memset(zbias, 0.0)

            ctx_sb = ctxpool.tile([HPC * DH, B * T], ctx_dt)
            pending_wo = []

            def _wo_block(j, b):
                for r in range(j * 4, (j + 1) * 4):
                    colr = b * T + r * 128
                    for ob in range(MODEL_DIM // 512):
                        wop = wopool.tile([128, 512], F32, tag="wop")
                        nc.tensor.matmul(
                            wop,
                            lhsT=ctx_sb[:, colr : colr + 128],
                            rhs=woT_sb[:, ob * 512 : (ob + 1) * 512],
                            start=True,
                            stop=True,
                        )
                        ot = outpool.tile([128, 512], F32, tag="ot")
                        nc.vector.tensor_copy(ot, wop)
                        nc.sync.dma_start(
                            out=out_d[
                                b * T + r * 128 : b * T + (r + 1) * 128,
                                ob * 512 : (ob + 1) * 512,
                            ],
                            in_=ot,
                        )

            def _load_batch(b):
                # first slices are small so the first S-matmul starts early
                qT_sb = io.tile([HPC * DH, T], qkv_dt, tag="qT", name=f"qT{b}")
                kT_sb = io.tile([HPC * DH, T], qkv_dt, tag="kT", name=f"kT{b}")
                splits = [0, T // 4, T // 2, T] if b == 0 else [0, T]
                for lo2, hi2 in zip(splits[:-1], splits[1:]):
                    cols = slice(lo2, hi2)
                    nc.sync.dma_start(out=kT_sb[:, cols], in_=kT_d[b][:, cols])
                    nc.sync.dma_start(out=qT_sb[:, cols], in_=qT_d[b][:, cols])
                vp_sb = io.tile(
                    [128, HPC * (T // 128), DH + 1], qkv_dt, tag="vp", name=f"vp{b}"
                )
                if b == 0:
                    # head0's first s-tiles land first so PV(j=0) starts early
                    nc.sync.dma_start(out=vp_sb[:, 0:4, :], in_=vp_d[b][:, 0:4, :])
                    nc.sync.dma_start(out=tri_sb, in_=tri_d)
                    nc.sync.dma_start(out=vp_sb[:, 4:, :], in_=vp_d[b][:, 4:, :])
                    nc.sync.dma_start(out=woT_sb, in_=woT_d)
                else:
                    nc.sync.dma_start(out=vp_sb, in_=vp_d[b])
                return qT_sb, kT_sb, vp_sb

            loaded = {0: _load_batch(0)}
            for b in range(B):
                qT_sb, kT_sb, vp_sb = loaded.pop(b)

                for j in range(n_tblk):
                    for hl in range(HPC):
                        bp = hl * DH  # partition base of this head's qT/kT rows
                        final_j = hl == HPC - 1 and b == B - 1 and j == n_tblk - 1
                        pv = pvpool.tile([DH + 1, TBLK], F32, tag="pv")
                        n_stiles = (j + 1) * (TBLK // STILE)
                        for c in range(0, n_stiles, CHUNK):
                            if final_j and c == CHUNK:
                                # flush deferred projections behind the final
                                # block's first S-chunk so ACT isn't starved
                                while pending_wo:
                                    pending_wo.pop(0)()
                            diag0 = j * (TBLK // STILE)  # first diagonal s-tile
                            # deep-diagonal chunk (tiles >= half-masked): narrow
                            # the S matmul + exp to the causally needed columns
                            deep = c >= diag0 + 2
                            sps = spspool.tile([128, CHUNK * TBLK], F32, tag="sps")
                            for m in range(CHUNK):
                                k = c + m
                                lo = (k - diag0) * STILE if deep else 0
                                nc.tensor.matmul(
                                    sps[:, m * TBLK + lo : (m + 1) * TBLK],
                                    lhsT=kT_sb[
                                        bp : bp + DH, k * STILE : (k + 1) * STILE
                                    ],
                                    rhs=qT_sb[
                                        bp : bp + DH, j * TBLK + lo : (j + 1) * TBLK
                                    ],
                                    start=True,
                                    stop=True,
                                    tile_position=(bp, 0),
                                )
                            ex = expool.tile([128, CHUNK * TBLK], ex_dt, tag="ex")
                            if deep:
                                for m in range(CHUNK):
                                    k = c + m
                                    lo = (k - diag0) * STILE
                                    nc.scalar.activation(
                                        ex[:, m * TBLK + lo : (m + 1) * TBLK],
                                        sps[:, m * TBLK + lo : (m + 1) * TBLK],
                                        func=mybir.ActivationFunctionType.Exp,
                                        bias=zbias,
                                        scale=0.125,
                                    )
                            else:
                                nc.scalar.activation(
                                    ex,
                                    sps,
                                    func=mybir.ActivationFunctionType.Exp,
                                    bias=zbias,
                                    scale=0.125,
                                )
                            for m in range(CHUNK):
                                k = c + m
                                if k >= diag0:
                                    # boundary 128-col gets the triangular mask;
                                    # fully-masked cols to its left are never
                                    # consumed (PV below starts at the boundary)
                                    mm = k - diag0
                                    exs = ex[:, m * TBLK : (m + 1) * TBLK]
                                    nc.gpsimd.tensor_mul(
                                        exs[:, mm * STILE : (mm + 1) * STILE],
                                        exs[:, mm * STILE : (mm + 1) * STILE],
                                        tri_sb,
                                    )
                            for m in range(CHUNK):
                                k = c + m
                                lo = (k - diag0) * STILE if k > diag0 else 0
                                nc.tensor.matmul(
                                    pv[:, lo:],
                                    lhsT=vp_sb[:, hl * (T // 128) + k, :],
                                    rhs=ex[:, m * TBLK + lo : (m + 1) * TBLK],
                                    start=(k == 0),
                                    stop=(k == n_stiles - 1),
                                    skip_group_check=True,
                                )

                        col = b * T + j * TBLK
                        last_block = (
                            hl == HPC - 1 and b == B - 1 and j == n_tblk - 1
                        )
                        if not last_block:
                            # normalize: ctx^T[d, t] = pv[d, t] / pv[64, t]
                            rc = normpool.tile([1, TBLK], F32, tag="rc")
                            nc.vector.reciprocal(rc, pv[DH : DH + 1, :])
                            rcb = normpool.tile([DH, TBLK], F32, tag="rcb")
                            nc.gpsimd.partition_broadcast(out_ap=rcb, in_ap=rc)
                            nc.vector.tensor_mul(
                                ctx_sb[hl * DH : (hl + 1) * DH, col : col + TBLK],
                                pv[0:DH, :],
                                rcb,
                            )
                        if hl == 0 and j == n_tblk - 1 and b + 1 < B:
                            # prefetch next batch's inputs now: their SP triggers
                            # queue ahead of this batch's output-DMA sem-waits
                            loaded[b + 1] = _load_batch(b + 1)
                        if hl == HPC - 1 and not last_block:
                            # emit the projection AFTER the next j's attention so
                            # the PE prioritizes S-matmuls (ACT's dependency)
                            def _emit_wo(j=j, b=b):
                                _wo_block(j, b)

                            pending_wo.append(_emit_wo)
                            if len(pending_wo) > 1:
                                pending_wo.pop(0)()
                        if last_block:
                            # drain deferred projections, then finish the final
                            # block at 128-col granularity so the tail pipeline
                            # (normalize -> Wo -> copy -> DMA) is 4x finer
                            while pending_wo:
                                pending_wo.pop(0)()
                            for pc in range(TBLK // STILE):
                                psl = slice(pc * STILE, (pc + 1) * STILE)
                                cp = col + pc * STILE
                                rc = normpool.tile([1, STILE], F32, tag="rcp")
                                nc.vector.reciprocal(rc, pv[DH : DH + 1, psl])
                                rcb = normpool.tile([DH, STILE], F32, tag="rcbp")
                                nc.gpsimd.partition_broadcast(out_ap=rcb, in_ap=rc)
                                nc.vector.tensor_mul(
                                    ctx_sb[hl * DH : (hl + 1) * DH, cp : cp + STILE],
                                    pv[0:DH, psl],
                                    rcb,
                                )
                                r = j * 4 + pc
                                for ob in range(MODEL_DIM // 512):
                                    wop = wopool.tile([128, 512], F32, tag="wop")
                                    nc.tensor.matmul(
                                        wop,
                                        lhsT=ctx_sb[:, cp : cp + STILE],
                                        rhs=woT_sb[:, ob * 512 : (ob + 1) * 512],
                                        start=True,
                                        stop=True,
                                    )
                                    ot = outpool.tile([128, 512], F32, tag="ot")
                                    if ob % 2 == 1:
                                        nc.scalar.copy(ot, wop)
                                    else:
                                        nc.vector.tensor_copy(ot, wop)
                                    nc.sync.dma_start(
                                        out=out_d[
                                            b * T + r * 128 : b * T + (r + 1) * 128,
                                            ob * 512 : (ob + 1) * 512,
                                        ],
                                        in_=ot,
                                    )

            for f in pending_wo:
                f()

    nc.compile()
    return nc


def make_core_inputs(q, k, v, Wo_w, mode=MODE):
    """Host-side sharding: per-core input dict list."""
    if mode == "bf16":
        import ml_dtypes

        io_dt = ml_dtypes.bfloat16
    else:
        io_dt = np.float32
    q = np.asarray(q, dtype=np.float32)
    k = np.asarray(k, dtype=np.float32)
    v = np.asarray(v, dtype=np.float32)
    Wo_w = np.asarray(Wo_w, dtype=np.float32)
    # multiplicative causal mask for a boundary tile: allow[s, t] = (s <= t)
    tri = np.triu(np.ones((STILE, STILE), dtype=io_dt), 0)
    in_maps = []
    for c in range(N_CORES):
        qT = np.empty((B, HPC * DH, T), dtype=io_dt)
        kT = np.empty((B, HPC * DH, T), dtype=io_dt)
        vp = np.empty((B, 128, HPC * (T // 128), DH + 1), dtype=io_dt)
        for b in range(B):
            for hl in range(HPC):
                h = HPC * c + hl
                qT[b, hl * DH : (hl + 1) * DH] = q[h, b].T
                kT[b, hl * DH : (hl + 1) * DH] = k[h, b].T
                vr = v[h, b].reshape(T // 128, 128, DH).transpose(1, 0, 2)
                sl = slice(hl * (T // 128), (hl + 1) * (T // 128))
                vp[b, :, sl, :DH] = vr
                vp[b, :, sl, DH] = 1.0
        woT = np.ascontiguousarray(Wo_w[:, 128 * c : 128 * (c + 1)].T)
        in_maps.append(
            {"qT": qT, "kT": kT, "vp": vp, "woT": woT, "tri": tri.copy()}
        )
    return in_maps


_NC_CACHE = {}


def get_program(mode=MODE):
    if mode not in _NC_CACHE:
        _NC_CACHE[mode] = _build_program(mode)
    return _NC_CACHE[mode]


def kernel(q, k, v, Wo_w, Wo_b, _trace=False, _trace_kwargs=None, mode=MODE):
    in_maps = make_core_inputs(q, k, v, Wo_w, mode)
    nc = get_program(mode)
    res = bass_utils.run_bass_kernel_spmd(
        nc,
        in_maps,
        core_ids=list(range(N_CORES)),
        trace=_trace,
        **(_trace_kwargs or {}),
    )
    total = np.zeros((B * T, MODEL_DIM), dtype=np.float32)
    for r in res.results:
        total += r["out"]
    total += np.asarray(Wo_b, dtype=np.float32)[None, :]
    out = total.reshape(B, T, MODEL_DIM)
    if _trace:
        return out, res
    return out


# revision 48
# speedup vs baseline: 1.0541x; 1.0202x over previous
"""Trainium2 Bass kernel for 16-head causal MHA + output projection.

Sharding: tensor-parallel over the head axis. 16 heads / 8 cores = 2 heads
per core. Each core runs causal flash-style attention for its 2 heads fully
on-chip, then multiplies its 128-wide slice of the concatenated context with
the matching 128 columns of Wo (row-parallel linear). The host gather sums
the 8 partial projections (the TP all-reduce realized as the unshard step)
and adds the bias.

Device algorithm per (head, batch) pair, per 512-wide query block j:
  S^T[s, t]   = sum_d K[s,d] Q[t,d]                  (PE, K=64 contraction)
  E           = exp(S^T * 0.125)                      (ACT, direct from PSUM;
                 no max subtraction -- scores are ~N(0,1), max ~6)
  causal mask: E[s,t] = 0 where s > t                 (0/1 triangular multiply
                 on boundary 128-tiles; fully-masked regions are simply never
                 computed/consumed)
  ctx^T[d, t] = sum_s V'[s,d] E[s,t]                  (PE, K=128; V' has a
                 ones column so row 64 of the output is the softmax denom)
  ctx^T[0:64] /= denom                                (DVE reciprocal + mul)
Then out_partial[t, :] = ctx^T[:, t].T @ WoT          (PE, K=128)

Q^T, K^T, V' and Wo^T slices are prepared on the host so the device never
transposes anything.
"""

import os
import sys

import numpy as np

try:
    import concourse.bass as bass  # noqa: F401
except ImportError:
    sys.path.insert(0, "/opt/trn_rl_repo")

import concourse.bass as bass
import concourse.tile as tile
from concourse import bacc, bass_utils, mybir

# Problem constants (hardcoded per contract)
H, B, T, DH = 16, 2, 2048, 64
MODEL_DIM = 1024
N_CORES = 8
HPC = H // N_CORES  # heads per core = 2
NPAIRS = HPC * B  # (head, batch) pairs per core = 4
TBLK = 512  # query-block width
STILE = 128  # key-block (s) tile height
CHUNK = 2  # s-tiles per PSUM chunk / exp instruction

F32 = mybir.dt.float32
F32R = mybir.dt.float32r
BF16 = mybir.dt.bfloat16

# "f32"  : everything fp32 (PE runs fp32 matmul at 1/4 rate — slow, exact)
# "f32r" : fp32 storage, matmuls issued as float32r (full rate, reduced mul precision)
# "bf16" : q/k/v/expS in bf16 (full rate, halved DMA), Wo matmul as f32r
MODE = os.environ.get("MHA_MODE", "f32r")


def _build_program(mode=MODE):
    nc = bacc.Bacc(
        "TRN2",
        debug=False,
        num_devices=N_CORES,
        name="mha_tp",
    )

    if mode == "bf16":
        qkv_dt = ex_dt = ctx_dt = BF16
    elif mode == "f32r":
        qkv_dt = ex_dt = ctx_dt = F32R
    else:
        qkv_dt = ex_dt = ctx_dt = F32

    # pair-packed layouts: group g = batch b holds head0 in partitions 0-63,
    # head1 in partitions 64-127 (full-partition DMAs run 2x faster)
    qT_d = nc.dram_tensor("qT", [B, HPC * DH, T], qkv_dt, kind="ExternalInput").ap()
    kT_d = nc.dram_tensor("kT", [B, HPC * DH, T], qkv_dt, kind="ExternalInput").ap()
    vp_d = nc.dram_tensor(
        "vp", [B, 128, HPC * (T // 128), DH + 1], qkv_dt, kind="ExternalInput"
    ).ap()
    woT_d = nc.dram_tensor(
        "woT", [HPC * DH, MODEL_DIM], ctx_dt, kind="ExternalInput"
    ).ap()
    tri_d = nc.dram_tensor("tri", [STILE, STILE], ex_dt, kind="ExternalInput").ap()
    out_d = nc.dram_tensor("out", [B * T, MODEL_DIM], F32, kind="ExternalOutput").ap()

    n_tblk = T // TBLK  # 4

    with tile.TileContext(nc) as tc:
        with (
            tc.tile_pool(name="const", bufs=1) as const,
            tc.tile_pool(name="io", bufs=2) as io,
            tc.tile_pool(name="exp", bufs=6) as expool,
            tc.tile_pool(name="ctx", bufs=1) as ctxpool,
            tc.tile_pool(name="norm", bufs=4) as normpool,
            tc.tile_pool(name="outp", bufs=6) as outpool,
            tc.tile_pool(name="sps", bufs=2, space="PSUM") as spspool,
            tc.tile_pool(name="pvp", bufs=2, space="PSUM") as pvpool,
            tc.tile_pool(name="wop", bufs=2, space="PSUM") as wopool,
        ):
            woT_sb = const.tile([HPC * DH, MODEL_DIM], ctx_dt)
            tri_sb = const.tile([STILE, STILE], ex_dt)
            zbias = const.tile([128, 1], F32)
            nc.vector.memset(zbias, 0.0)

            ctx_sb = ctxpool.tile([HPC * DH, B * T], ctx_dt)
            pending_wo = []

            def _wo_block(j, b):
                for r in range(j * 4, (j + 1) * 4):
                    colr = b * T + r * 128
                    for ob in range(MODEL_DIM // 512):
                        wop = wopool.tile([128, 512], F32, tag="wop")
                        nc.tensor.matmul(
                            wop,
                            lhsT=ctx_sb[:, colr : colr + 128],
                            rhs=woT_sb[:, ob * 512 : (ob + 1) * 512],
                            start=True,
                            stop=True,
                        )
                        ot = outpool.tile([128, 512], F32, tag="ot")
                        nc.vector.tensor_copy(ot, wop)
                        nc.sync.dma_start(
                            out=out_d[
                                b * T + r * 128 : b * T + (r + 1) * 128,
                                ob * 512 : (ob + 1) * 512,
                            ],
                            in_=ot,
                        )

            def _load_batch(b):
                # first slices are small so the first S-matmul starts early
                qT_sb = io.tile([HPC * DH, T], qkv_dt, tag="qT", name=f"qT{b}")
                kT_sb = io.tile([HPC * DH, T], qkv_dt, tag="kT", name=f"kT{b}")
                splits = [0, T // 4, T // 2, T] if b == 0 else [0, T]
                for lo2, hi2 in zip(splits[:-1], splits[1:]):
                    cols = slice(lo2, hi2)
                    nc.sync.dma_start(out=kT_sb[:, cols], in_=kT_d[b][:, cols])
                    nc.sync.dma_start(out=qT_sb[:, cols], in_=qT_d[b][:, cols])
                vp_sb = io.tile(
                    [128, HPC * (T // 128), DH + 1], qkv_dt, tag="vp", name=f"vp{b}"
                )
                if b == 0:
                    # head0's first s-tiles land first so PV(j=0) starts early
                    nc.sync.dma_start(out=vp_sb[:, 0:4, :], in_=vp_d[b][:, 0:4, :])
                    nc.sync.dma_start(out=tri_sb, in_=tri_d)
                    nc.sync.dma_start(out=vp_sb[:, 4:, :], in_=vp_d[b][:, 4:, :])
                    nc.sync.dma_start(out=woT_sb, in_=woT_d)
                else:
                    nc.sync.dma_start(out=vp_sb, in_=vp_d[b])
                return qT_sb, kT_sb, vp_sb

            loaded = {0: _load_batch(0)}
            for b in range(B):
                qT_sb, kT_sb, vp_sb = loaded.pop(b)

                for j in range(n_tblk):
                    for hl in range(HPC):
                        bp = hl * DH  # partition base of this head's qT/kT rows
                        final_j = hl == HPC - 1 and b == B - 1 and j == n_tblk - 1
                        pv = pvpool.tile([DH + 1, TBLK], F32, tag="pv")
                        n_stiles = (j + 1) * (TBLK // STILE)
                        for c in range(0, n_stiles, CHUNK):
                            if final_j and c == 2 * CHUNK:
                                # flush deferred projections behind the final
                                # block's first S-chunk so ACT isn't starved
                                while pending_wo:
                                    pending_wo.pop(0)()
                            diag0 = j * (TBLK // STILE)  # first diagonal s-tile
                            # deep-diagonal chunk (tiles >= half-masked): narrow
                            # the S matmul + exp to the causally needed columns
                            deep = c >= diag0 + 2
                            sps = spspool.tile([128, CHUNK * TBLK], F32, tag="sps")
                            for m in range(CHUNK):
                                k = c + m
                                lo = (k - diag0) * STILE if deep else 0
                                nc.tensor.matmul(
                                    sps[:, m * TBLK + lo : (m + 1) * TBLK],
                                    lhsT=kT_sb[
                                        bp : bp + DH, k * STILE : (k + 1) * STILE
                                    ],
                                    rhs=qT_sb[
                                        bp : bp + DH, j * TBLK + lo : (j + 1) * TBLK
                                    ],
                                    start=True,
                                    stop=True,
                                    tile_position=(bp, 0),
                                )
                            ex = expool.tile([128, CHUNK * TBLK], ex_dt, tag="ex")
                            if deep:
                                for m in range(CHUNK):
                                    k = c + m
                                    lo = (k - diag0) * STILE
                                    nc.scalar.activation(
                                        ex[:, m * TBLK + lo : (m + 1) * TBLK],
                                        sps[:, m * TBLK + lo : (m + 1) * TBLK],
                                        func=mybir.ActivationFunctionType.Exp,
                                        bias=zbias,
                                        scale=0.125,
                                    )
                            else:
                                nc.scalar.activation(
                                    ex,
                                    sps,
                                    func=mybir.ActivationFunctionType.Exp,
                                    bias=zbias,
                                    scale=0.125,
                                )
                            for m in range(CHUNK):
                                k = c + m
                                if k >= diag0:
                                    # boundary 128-col gets the triangular mask;
                                    # fully-masked cols to its left are never
                                    # consumed (PV below starts at the boundary)
                                    mm = k - diag0
                                    exs = ex[:, m * TBLK : (m + 1) * TBLK]
                                    nc.gpsimd.tensor_mul(
                                        exs[:, mm * STILE : (mm + 1) * STILE],
                                        exs[:, mm * STILE : (mm + 1) * STILE],
                                        tri_sb,
                                    )
                            for m in range(CHUNK):
                                k = c + m
                                lo = (k - diag0) * STILE if k > diag0 else 0
                                nc.tensor.matmul(
                                    pv[:, lo:],
                                    lhsT=vp_sb[:, hl * (T // 128) + k, :],
                                    rhs=ex[:, m * TBLK + lo : (m + 1) * TBLK],
                                    start=(k == 0),
                                    stop=(k == n_stiles - 1),
                                    skip_group_check=True,
                                )

                        col = b * T + j * TBLK
                        last_block = (
                            hl == HPC - 1 and b == B - 1 and j == n_tblk - 1
                        )
                        if not last_block:
                            # normalize: ctx^T[d, t] = pv[d, t] / pv[64, t]
                            rc = normpool.tile([1, TBLK], F32, tag="rc")
                            nc.vector.reciprocal(rc, pv[DH : DH + 1, :])
                            rcb = normpool.tile([DH, TBLK], F32, tag="rcb")
                            nc.gpsimd.partition_broadcast(out_ap=rcb, in_ap=rc)
                            nc.vector.tensor_mul(
                                ctx_sb[hl * DH : (hl + 1) * DH, col : col + TBLK],
                                pv[0:DH, :],
                                rcb,
                            )
                        if hl == 0 and j == n_tblk - 1 and b + 1 < B:
                            # prefetch next batch's inputs now: their SP triggers
                            # queue ahead of this batch's output-DMA sem-waits
                            loaded[b + 1] = _load_batch(b + 1)
                        if hl == HPC - 1 and not last_block:
                            # emit the projection AFTER the next j's attention so
                            # the PE prioritizes S-matmuls (ACT's dependency)
                            def _emit_wo(j=j, b=b):
                                _wo_block(j, b)

                            pending_wo.append(_emit_wo)
                            if len(pending_wo) > 3:
                                pending_wo.pop(0)()
                        if last_block:
                            # drain deferred projections, then finish the final
                            # block at 128-col granularity so the tail pipeline
                            # (normalize -> Wo -> copy -> DMA) is 4x finer
                            while pending_wo:
                                pending_wo.pop(0)()
                            for pc in range(TBLK // STILE):
                                psl = slice(pc * STILE, (pc + 1) * STILE)
                                cp = col + pc * STILE
                                rc = normpool.tile([1, STILE], F32, tag="rcp")
                                nc.vector.reciprocal(rc, pv[DH : DH + 1, psl])
                                rcb = normpool.tile([DH, STILE], F32, tag="rcbp")
                                nc.gpsimd.partition_broadcast(out_ap=rcb, in_ap=rc)
                                nc.vector.tensor_mul(
                                    ctx_sb[hl * DH : (hl + 1) * DH, cp : cp + STILE],
                                    pv[0:DH, psl],
                                    rcb,
                                )
                                r = j * 4 + pc
                                for ob in range(MODEL_DIM // 512):
                                    wop = wopool.tile([128, 512], F32, tag="wop")
                                    nc.tensor.matmul(
                                        wop,
                                        lhsT=ctx_sb[:, cp : cp + STILE],
                                        rhs=woT_sb[:, ob * 512 : (ob + 1) * 512],
                                        start=True,
                                        stop=True,
                                    )
                                    ot = outpool.tile([128, 512], F32, tag="ot")
                                    if ob % 2 == 1:
                                        nc.scalar.copy(ot, wop)
                                    else:
                                        nc.vector.tensor_copy(ot, wop)
                                    nc.sync.dma_start(
                                        out=out_d[
                                            b * T + r * 128 : b * T + (r + 1) * 128,
                                            ob * 512 : (ob + 1) * 512,
                                        ],
                                        in_=ot,
                                    )

            for f in pending_wo:
                f()

    nc.compile()
    return nc


def make_core_inputs(q, k, v, Wo_w, mode=MODE):
    """Host-side sharding: per-core input dict list."""
    if mode == "bf16":
        import ml_dtypes

        io_dt = ml_dtypes.bfloat16
    else:
        io_dt = np.float32
    q = np.asarray(q, dtype=np.float32)
    k = np.asarray(k, dtype=np.float32)
    v = np.asarray(v, dtype=np.float32)
    Wo_w = np.asarray(Wo_w, dtype=np.float32)
    # multiplicative causal mask for a boundary tile: allow[s, t] = (s <= t)
    tri = np.triu(np.ones((STILE, STILE), dtype=io_dt), 0)
    in_maps = []
    for c in range(N_CORES):
        qT = np.empty((B, HPC * DH, T), dtype=io_dt)
        kT = np.empty((B, HPC * DH, T), dtype=io_dt)
        vp = np.empty((B, 128, HPC * (T // 128), DH + 1), dtype=io_dt)
        for b in range(B):
            for hl in range(HPC):
                h = HPC * c + hl
                qT[b, hl * DH : (hl + 1) * DH] = q[h, b].T
                kT[b, hl * DH : (hl + 1) * DH] = k[h, b].T
                vr = v[h, b].reshape(T // 128, 128, DH).transpose(1, 0, 2)
                sl = slice(hl * (T // 128), (hl + 1) * (T // 128))
                vp[b, :, sl, :DH] = vr
                vp[b, :, sl, DH] = 1.0
        woT = np.ascontiguousarray(Wo_w[:, 128 * c : 128 * (c + 1)].T)
        in_maps.append(
            {"qT": qT, "kT": kT, "vp": vp, "woT": woT, "tri": tri.copy()}
        )
    return in_maps


_NC_CACHE = {}


def get_program(mode=MODE):
    if mode not in _NC_CACHE:
        _NC_CACHE[mode] = _build_program(mode)
    return _NC_CACHE[mode]


def kernel(q, k, v, Wo_w, Wo_b, _trace=False, _trace_kwargs=None, mode=MODE):
    in_maps = make_core_inputs(q, k, v, Wo_w, mode)
    nc = get_program(mode)
    res = bass_utils.run_bass_kernel_spmd(
        nc,
        in_maps,
        core_ids=list(range(N_CORES)),
        trace=_trace,
        **(_trace_kwargs or {}),
    )
    total = np.zeros((B * T, MODEL_DIM), dtype=np.float32)
    for r in res.results:
        total += r["out"]
    total += np.asarray(Wo_b, dtype=np.float32)[None, :]
    out = total.reshape(B, T, MODEL_DIM)
    if _trace:
        return out, res
    return out
